# revision 5
# baseline (speedup 1.0000x reference)
"""Non-local block (no softmax) on 8 Trainium2 cores, data-parallel over batch.

Math: per sample X [N=4096, C=256] (N = 64*64 spatial, C channels):
    theta = X Wt, phi = X Wp, g = X Wg          (biases are zero)
    y = (theta phi^T / N) g  ->  associativity (no softmax):
      y = X Wt A / N,  A = Wp^T G Wg,  G = X^T X
    z = y (Ww * s) + t2 + X  =  X (M2 + I) + t2
      M2 = Wt (A/N) (Ww * s),  s = gamma*rsqrt(var+eps),
      t2 = (b_W - mean)*s + beta
so the whole block is: G = X^T X (device), small 256x256 chain (device),
z = X (M2+I) + t2 (device). One sample per NeuronCore.
"""

import numpy as np

B, H, W, C = 8, 64, 64, 256
IC = C // 2
N = H * W
NCHUNK = N // 128  # 32
BN_EPS = 1e-3

_CACHE = {}


def _build_nc(f32r_big: bool = True):
    import concourse.bacc as bacc
    import concourse.mybir as mybir
    import concourse.tile as tile

    F32 = mybir.dt.float32
    F32R = mybir.dt.float32r

    def rc(ap):
        # round-cast view: tiles written through this view carry the f32r
        # dtype the walrus verifier demands of fp32r-matmul producers
        return ap.bitcast(F32R) if f32r_big else ap

    nc = bacc.Bacc("TRN2", target_bir_lowering=False, debug=False)

    x_d = nc.dram_tensor("x", [N, C], F32, kind="ExternalInput")
    wphi_d = nc.dram_tensor("wphi", [128, 256], F32, kind="ExternalInput")
    wg_d = nc.dram_tensor("wg", [128, 256], F32, kind="ExternalInput")
    wtt_d = nc.dram_tensor("wtt", [128, 256], F32, kind="ExternalInput")
    wwf_d = nc.dram_tensor("wwf", [128, 256], F32, kind="ExternalInput")
    t2b_d = nc.dram_tensor("t2b", [128, 256], F32, kind="ExternalInput")
    id2_d = nc.dram_tensor("id2", [128, 512], F32, kind="ExternalInput")
    z_d = nc.dram_tensor("z", [N, C], F32, kind="ExternalOutput")

    with tile.TileContext(nc) as tc:
        with (
            tc.tile_pool(name="const", bufs=1) as cpool,
            tc.tile_pool(name="big", bufs=1) as bpool,
            tc.tile_pool(name="zs", bufs=4) as zpool,
            tc.tile_pool(name="psg", bufs=1, space="PSUM") as psg,
            tc.tile_pool(name="psw", bufs=4, space="PSUM") as psw,
        ):
            wphi = cpool.tile([128, 256], F32, tag="wphi")
            wg = cpool.tile([128, 256], F32, tag="wg")
            wtt = cpool.tile([128, 256], F32, tag="wtt")
            wwf = cpool.tile([128, 256], F32, tag="wwf")
            t2b = cpool.tile([128, 256], F32, tag="t2b")
            id2 = cpool.tile([128, 512], F32, tag="id2")
            nc.gpsimd.dma_start(rc(wphi[:]), rc(wphi_d[:]))
            nc.gpsimd.dma_start(rc(wg[:]), rc(wg_d[:]))
            nc.gpsimd.dma_start(rc(wtt[:]), rc(wtt_d[:]))
            nc.gpsimd.dma_start(rc(wwf[:]), rc(wwf_d[:]))
            nc.gpsimd.dma_start(t2b[:], t2b_d[:])
            nc.gpsimd.dma_start(rc(id2[:]), rc(id2_d[:]))

            x_nat = bpool.tile([128, NCHUNK * 256], F32, tag="x_nat")
            xt0 = bpool.tile([128, N], F32, tag="xt0")
            xt1 = bpool.tile([128, N], F32, tag="xt1")

            ident = id2[:, 0:128]  # eye(128)

            # ---- phase 1: load x, G = X^T X, and build X^T via PE transpose
            g0 = psg.tile([128, 256], F32, tag="g0")
            g1 = psg.tile([128, 256], F32, tag="g1")
            for t in range(NCHUNK):
                xc = x_nat[:, t * 256 : (t + 1) * 256]
                xcl = x_nat[:, t * 256 : t * 256 + 128]
                xch = x_nat[:, t * 256 + 128 : (t + 1) * 256]
                dma_eng = nc.sync if t % 2 == 0 else nc.scalar
                dma_eng.dma_start(rc(xc), rc(x_d[t * 128 : (t + 1) * 128, :]))
                nc.tensor.matmul(
                    g0[:], rc(xcl), rc(xc),
                    start=(t == 0), stop=(t == NCHUNK - 1),
                )
                nc.tensor.matmul(
                    g1[:], rc(xch), rc(xc),
                    start=(t == 0), stop=(t == NCHUNK - 1),
                )
                tp0 = psw.tile([128, 128], F32, tag="w")
                nc.tensor.transpose(rc(tp0[:]), rc(xcl), rc(ident))
                tp1 = psw.tile([128, 128], F32, tag="w")
                nc.tensor.transpose(rc(tp1[:]), rc(xch), rc(ident))
                dst0 = rc(xt0[:, t * 128 : (t + 1) * 128])
                dst1 = rc(xt1[:, t * 128 : (t + 1) * 128])
                # split PSUM->SBUF copies across DVE and ACT
                if t % 2 == 0:
                    nc.vector.tensor_copy(dst0, rc(tp0[:]))
                    nc.scalar.copy(dst1, rc(tp1[:]))
                else:
                    nc.scalar.copy(dst0, rc(tp0[:]))
                    nc.vector.tensor_copy(dst1, rc(tp1[:]))

            # ---- phase 2: small chain
            g_s = bpool.tile([128, 512], F32, tag="g_s")
            nc.vector.tensor_copy(rc(g_s[:, 0:256]), g0[:])
            nc.vector.tensor_copy(rc(g_s[:, 256:512]), g1[:])

            pp1 = psw.tile([128, 256], F32, tag="w")
            nc.tensor.matmul(pp1[:], rc(wphi[:, 0:128]), rc(g_s[:, 0:256]),
                             start=True, stop=False)
            nc.tensor.matmul(pp1[:], rc(wphi[:, 128:256]), rc(g_s[:, 256:512]),
                             start=False, stop=True)
            p1_s = bpool.tile([128, 256], F32, tag="p1_s")
            nc.vector.tensor_copy(rc(p1_s[:]), pp1[:])

            p1t_s = bpool.tile([128, 256], F32, tag="p1t_s")
            for j in range(2):
                tp = psw.tile([128, 128], F32, tag="w")
                nc.tensor.transpose(rc(tp[:]),
                                    rc(p1_s[:, j * 128 : (j + 1) * 128]),
                                    rc(ident))
                nc.vector.tensor_copy(rc(p1t_s[:, j * 128 : (j + 1) * 128]),
                                      rc(tp[:]))

            pa = psw.tile([128, 128], F32, tag="w")
            nc.tensor.matmul(pa[:], rc(p1t_s[:, 0:128]), rc(wg[:, 0:128]),
                             start=True, stop=False)
            nc.tensor.matmul(pa[:], rc(p1t_s[:, 128:256]), rc(wg[:, 128:256]),
                             start=False, stop=True)
            a_s = bpool.tile([128, 128], F32, tag="a_s")
            nc.vector.tensor_copy(rc(a_s[:]), pa[:])

            t1_s = bpool.tile([128, 256], F32, tag="t1_s")
            for j in range(2):
                pt = psw.tile([128, 128], F32, tag="w")
                nc.tensor.matmul(pt[:], rc(wtt[:, j * 128 : (j + 1) * 128]),
                                 rc(a_s[:]), start=True, stop=True)
                nc.vector.tensor_copy(rc(t1_s[:, j * 128 : (j + 1) * 128]), pt[:])

            t1t_s = bpool.tile([128, 256], F32, tag="t1t_s")
            for j in range(2):
                tp = psw.tile([128, 128], F32, tag="w")
                nc.tensor.transpose(rc(tp[:]),
                                    rc(t1_s[:, j * 128 : (j + 1) * 128]),
                                    rc(ident))
                nc.vector.tensor_copy(rc(t1t_s[:, j * 128 : (j + 1) * 128]),
                                      rc(tp[:]))

            m2_s = bpool.tile([128, 512], F32, tag="m2_s")
            for j in range(2):
                pm = psw.tile([128, 256], F32, tag="w")
                nc.tensor.matmul(pm[:], rc(t1t_s[:, j * 128 : (j + 1) * 128]),
                                 rc(wwf[:]), start=True, stop=True)
                # M2' = M2 + I  (adds the residual path)
                nc.vector.tensor_add(rc(m2_s[:, j * 256 : (j + 1) * 256]), pm[:],
                                     id2[:, j * 256 : (j + 1) * 256])

            # ---- phase 3: z = X M2' + t2
            for t in range(NCHUNK):
                pz = psw.tile([128, 256], F32, tag="w")
                nc.tensor.matmul(pz[:], rc(xt0[:, t * 128 : (t + 1) * 128]),
                                 rc(m2_s[:, 0:256]), start=True, stop=False)
                nc.tensor.matmul(pz[:], rc(xt1[:, t * 128 : (t + 1) * 128]),
                                 rc(m2_s[:, 256:512]), start=False, stop=True)
                z_s = zpool.tile([128, 256], F32, tag="z")
                nc.vector.tensor_add(z_s[:], pz[:], t2b[:])
                (nc.sync if t % 2 == 0 else nc.scalar).dma_start(z_d[t * 128 : (t + 1) * 128, :], z_s[:])

    nc.compile()
    return nc


def _get_nc(f32r_big=True):
    key = ("nc", f32r_big)
    if key not in _CACHE:
        _CACHE[key] = _build_nc(f32r_big)
    return _CACHE[key]


def _fold_params(w_g, b_g, w_theta, b_theta, w_phi, b_phi, w_W, b_W,
                 bn_gamma, bn_beta, bn_mean, bn_var):
    f32 = np.float32
    s = (bn_gamma / np.sqrt(bn_var + BN_EPS)).astype(f32)
    t2 = ((b_W - bn_mean) * s + bn_beta).astype(f32)
    pack = lambda w: np.ascontiguousarray(
        np.concatenate([w[:128, :], w[128:, :]], axis=1), dtype=f32)
    wphi_p = pack(np.asarray(w_phi))
    wg_p = pack(np.asarray(w_g))
    wtt = np.ascontiguousarray(np.asarray(w_theta).T, dtype=f32)
    wwf = np.ascontiguousarray(np.asarray(w_W) * s[None, :] / N, dtype=f32)
    t2b = np.ascontiguousarray(np.broadcast_to(t2, (128, C)), dtype=f32)
    eye = np.eye(C, dtype=f32)
    id2 = np.ascontiguousarray(np.concatenate([eye[:128, :], eye[128:, :]], axis=1))
    return wphi_p, wg_p, wtt, wwf, t2b, id2


def _reference_fallback(x, w_g, b_g, w_theta, b_theta, w_phi, b_phi, w_W, b_W,
                        bn_gamma, bn_beta, bn_mean, bn_var):
    b, h, w, c = x.shape
    n = h * w
    xf = x.reshape(b, n, c).astype(np.float32)
    g_x = xf @ w_g + b_g
    theta_x = xf @ w_theta + b_theta
    phi_x = xf @ w_phi + b_phi
    a = np.einsum("bnd,bne->bde", phi_x, g_x) / n
    y = theta_x @ a
    w_y = y @ w_W + b_W
    w_y = bn_gamma * (w_y - bn_mean) / np.sqrt(bn_var + BN_EPS) + bn_beta
    return (w_y.reshape(b, h, w, c) + x).astype(np.float32)


def run_sharded(x, folded, f32r_big=True, trace=False):
    from concourse.bass_utils import run_bass_kernel_spmd

    nc = _get_nc(f32r_big)
    wphi_p, wg_p, wtt, wwf, t2b, id2 = folded
    xr = np.ascontiguousarray(np.asarray(x), dtype=np.float32).reshape(B, N, C)
    in_maps = [
        {"x": xr[i], "wphi": wphi_p, "wg": wg_p, "wtt": wtt, "wwf": wwf,
         "t2b": t2b, "id2": id2}
        for i in range(B)
    ]
    res = run_bass_kernel_spmd(nc, in_maps, list(range(B)), trace=trace)
    z = np.stack([res.results[i]["z"] for i in range(B)], axis=0)
    return z.reshape(B, H, W, C), res


def kernel(x, w_g, b_g, w_theta, b_theta, w_phi, b_phi, w_W, b_W,
           bn_gamma, bn_beta, bn_mean, bn_var):
    args = dict(w_g=np.asarray(w_g), b_g=np.asarray(b_g),
                w_theta=np.asarray(w_theta), b_theta=np.asarray(b_theta),
                w_phi=np.asarray(w_phi), b_phi=np.asarray(b_phi),
                w_W=np.asarray(w_W), b_W=np.asarray(b_W),
                bn_gamma=np.asarray(bn_gamma), bn_beta=np.asarray(bn_beta),
                bn_mean=np.asarray(bn_mean), bn_var=np.asarray(bn_var))
    x = np.asarray(x)
    # the device path folds the (zero) projection biases away; anything else
    # (never produced by setup_inputs) gets the exact host fallback
    if (np.any(args["b_g"]) or np.any(args["b_theta"]) or np.any(args["b_phi"])
            or x.shape != (B, H, W, C)):
        return _reference_fallback(x, **{k: v for k, v in args.items()})
    folded = _fold_params(**args)
    z, _ = run_sharded(x, folded)
    return z


# revision 6
# speedup vs baseline: 1.0174x; 1.0174x over previous
"""Non-local block (no softmax) on 8 Trainium2 cores, data-parallel over batch.

Math: per sample X [N=4096, C=256] (N = 64*64 spatial, C channels):
    theta = X Wt, phi = X Wp, g = X Wg          (biases are zero)
    y = (theta phi^T / N) g  ->  associativity (no softmax):
      y = X Wt A / N,  A = Wp^T G Wg,  G = X^T X
    z = y (Ww * s) + t2 + X  =  X (M2 + I) + t2
      M2 = Wt (A/N) (Ww * s),  s = gamma*rsqrt(var+eps),
      t2 = (b_W - mean)*s + beta
so the whole block is: G = X^T X (device), small 256x256 chain (device),
z = X (M2+I) + t2 (device). One sample per NeuronCore.
"""

import numpy as np

B, H, W, C = 8, 64, 64, 256
IC = C // 2
N = H * W
NCHUNK = N // 128  # 32
BN_EPS = 1e-3

_CACHE = {}


def _build_nc(f32r_big: bool = True):
    import concourse.bacc as bacc
    import concourse.mybir as mybir
    import concourse.tile as tile

    F32 = mybir.dt.float32
    F32R = mybir.dt.float32r

    def rc(ap):
        # round-cast view: tiles written through this view carry the f32r
        # dtype the walrus verifier demands of fp32r-matmul producers
        return ap.bitcast(F32R) if f32r_big else ap

    nc = bacc.Bacc("TRN2", target_bir_lowering=False, debug=False)

    x_d = nc.dram_tensor("x", [N, C], F32, kind="ExternalInput")
    wphi_d = nc.dram_tensor("wphi", [128, 256], F32, kind="ExternalInput")
    wg_d = nc.dram_tensor("wg", [128, 256], F32, kind="ExternalInput")
    wtt_d = nc.dram_tensor("wtt", [128, 256], F32, kind="ExternalInput")
    wwf_d = nc.dram_tensor("wwf", [128, 256], F32, kind="ExternalInput")
    t2b_d = nc.dram_tensor("t2b", [128, 256], F32, kind="ExternalInput")
    id2_d = nc.dram_tensor("id2", [128, 512], F32, kind="ExternalInput")
    z_d = nc.dram_tensor("z", [N, C], F32, kind="ExternalOutput")

    with tile.TileContext(nc) as tc:
        with (
            tc.tile_pool(name="const", bufs=1) as cpool,
            tc.tile_pool(name="big", bufs=1) as bpool,
            tc.tile_pool(name="zs", bufs=4) as zpool,
            tc.tile_pool(name="psg", bufs=1, space="PSUM") as psg,
            tc.tile_pool(name="psw", bufs=4, space="PSUM") as psw,
        ):
            wphi = cpool.tile([128, 256], F32, tag="wphi")
            wg = cpool.tile([128, 256], F32, tag="wg")
            wtt = cpool.tile([128, 256], F32, tag="wtt")
            wwf = cpool.tile([128, 256], F32, tag="wwf")
            t2b = cpool.tile([128, 256], F32, tag="t2b")
            id2 = cpool.tile([128, 512], F32, tag="id2")
            nc.sync.dma_start(rc(id2[:]), rc(id2_d[:]))

            x_nat = bpool.tile([128, NCHUNK * 256], F32, tag="x_nat")
            xt0 = bpool.tile([128, N], F32, tag="xt0")
            xt1 = bpool.tile([128, N], F32, tag="xt1")

            ident = id2[:, 0:128]  # eye(128)

            # ---- phase 1: load x, G = X^T X, and build X^T via PE transpose
            g0 = psg.tile([128, 256], F32, tag="g0")
            g1 = psg.tile([128, 256], F32, tag="g1")
            for t in range(NCHUNK):
                xc = x_nat[:, t * 256 : (t + 1) * 256]
                xcl = x_nat[:, t * 256 : t * 256 + 128]
                xch = x_nat[:, t * 256 + 128 : (t + 1) * 256]
                dma_eng = nc.sync if t % 2 == 0 else nc.gpsimd
                dma_eng.dma_start(rc(xc), rc(x_d[t * 128 : (t + 1) * 128, :]))
                if t == 3:
                    nc.gpsimd.dma_start(rc(wphi[:]), rc(wphi_d[:]))
                    nc.sync.dma_start(rc(wg[:]), rc(wg_d[:]))
                    nc.gpsimd.dma_start(rc(wtt[:]), rc(wtt_d[:]))
                    nc.sync.dma_start(rc(wwf[:]), rc(wwf_d[:]))
                    nc.gpsimd.dma_start(t2b[:], t2b_d[:])
                nc.tensor.matmul(
                    g0[:], rc(xcl), rc(xc),
                    start=(t == 0), stop=(t == NCHUNK - 1),
                )
                nc.tensor.matmul(
                    g1[:], rc(xch), rc(xc),
                    start=(t == 0), stop=(t == NCHUNK - 1),
                )
                tp0 = psw.tile([128, 128], F32, tag="w")
                nc.tensor.transpose(rc(tp0[:]), rc(xcl), rc(ident))
                tp1 = psw.tile([128, 128], F32, tag="w")
                nc.tensor.transpose(rc(tp1[:]), rc(xch), rc(ident))
                dst0 = rc(xt0[:, t * 128 : (t + 1) * 128])
                dst1 = rc(xt1[:, t * 128 : (t + 1) * 128])
                # split PSUM->SBUF copies across DVE and ACT
                if t % 2 == 0:
                    nc.vector.tensor_copy(dst0, rc(tp0[:]))
                    nc.scalar.copy(dst1, rc(tp1[:]))
                else:
                    nc.scalar.copy(dst0, rc(tp0[:]))
                    nc.vector.tensor_copy(dst1, rc(tp1[:]))

            # ---- phase 2: small chain
            g_s = bpool.tile([128, 512], F32, tag="g_s")
            nc.vector.tensor_copy(rc(g_s[:, 0:256]), g0[:])
            nc.vector.tensor_copy(rc(g_s[:, 256:512]), g1[:])

            pp1 = psw.tile([128, 256], F32, tag="w")
            nc.tensor.matmul(pp1[:], rc(wphi[:, 0:128]), rc(g_s[:, 0:256]),
                             start=True, stop=False)
            nc.tensor.matmul(pp1[:], rc(wphi[:, 128:256]), rc(g_s[:, 256:512]),
                             start=False, stop=True)
            p1_s = bpool.tile([128, 256], F32, tag="p1_s")
            nc.vector.tensor_copy(rc(p1_s[:]), pp1[:])

            p1t_s = bpool.tile([128, 256], F32, tag="p1t_s")
            for j in range(2):
                tp = psw.tile([128, 128], F32, tag="w")
                nc.tensor.transpose(rc(tp[:]),
                                    rc(p1_s[:, j * 128 : (j + 1) * 128]),
                                    rc(ident))
                nc.vector.tensor_copy(rc(p1t_s[:, j * 128 : (j + 1) * 128]),
                                      rc(tp[:]))

            pa = psw.tile([128, 128], F32, tag="w")
            nc.tensor.matmul(pa[:], rc(p1t_s[:, 0:128]), rc(wg[:, 0:128]),
                             start=True, stop=False)
            nc.tensor.matmul(pa[:], rc(p1t_s[:, 128:256]), rc(wg[:, 128:256]),
                             start=False, stop=True)
            a_s = bpool.tile([128, 128], F32, tag="a_s")
            nc.vector.tensor_copy(rc(a_s[:]), pa[:])

            t1_s = bpool.tile([128, 256], F32, tag="t1_s")
            for j in range(2):
                pt = psw.tile([128, 128], F32, tag="w")
                nc.tensor.matmul(pt[:], rc(wtt[:, j * 128 : (j + 1) * 128]),
                                 rc(a_s[:]), start=True, stop=True)
                nc.vector.tensor_copy(rc(t1_s[:, j * 128 : (j + 1) * 128]), pt[:])

            t1t_s = bpool.tile([128, 256], F32, tag="t1t_s")
            for j in range(2):
                tp = psw.tile([128, 128], F32, tag="w")
                nc.tensor.transpose(rc(tp[:]),
                                    rc(t1_s[:, j * 128 : (j + 1) * 128]),
                                    rc(ident))
                nc.vector.tensor_copy(rc(t1t_s[:, j * 128 : (j + 1) * 128]),
                                      rc(tp[:]))

            m2_s = bpool.tile([128, 512], F32, tag="m2_s")
            for j in range(2):
                pm = psw.tile([128, 256], F32, tag="w")
                nc.tensor.matmul(pm[:], rc(t1t_s[:, j * 128 : (j + 1) * 128]),
                                 rc(wwf[:]), start=True, stop=True)
                # M2' = M2 + I  (adds the residual path)
                nc.vector.tensor_add(rc(m2_s[:, j * 256 : (j + 1) * 256]), pm[:],
                                     id2[:, j * 256 : (j + 1) * 256])

            # ---- phase 3: z = X M2' + t2
            for t in range(NCHUNK):
                pz = psw.tile([128, 256], F32, tag="w")
                nc.tensor.matmul(pz[:], rc(xt0[:, t * 128 : (t + 1) * 128]),
                                 rc(m2_s[:, 0:256]), start=True, stop=False)
                nc.tensor.matmul(pz[:], rc(xt1[:, t * 128 : (t + 1) * 128]),
                                 rc(m2_s[:, 256:512]), start=False, stop=True)
                z_s = zpool.tile([128, 256], F32, tag="z")
                nc.vector.tensor_add(z_s[:], pz[:], t2b[:])
                (nc.sync if t % 2 == 0 else nc.gpsimd).dma_start(z_d[t * 128 : (t + 1) * 128, :], z_s[:])

    nc.compile()
    return nc


def _get_nc(f32r_big=True):
    key = ("nc", f32r_big)
    if key not in _CACHE:
        _CACHE[key] = _build_nc(f32r_big)
    return _CACHE[key]


def _fold_params(w_g, b_g, w_theta, b_theta, w_phi, b_phi, w_W, b_W,
                 bn_gamma, bn_beta, bn_mean, bn_var):
    f32 = np.float32
    s = (bn_gamma / np.sqrt(bn_var + BN_EPS)).astype(f32)
    t2 = ((b_W - bn_mean) * s + bn_beta).astype(f32)
    pack = lambda w: np.ascontiguousarray(
        np.concatenate([w[:128, :], w[128:, :]], axis=1), dtype=f32)
    wphi_p = pack(np.asarray(w_phi))
    wg_p = pack(np.asarray(w_g))
    wtt = np.ascontiguousarray(np.asarray(w_theta).T, dtype=f32)
    wwf = np.ascontiguousarray(np.asarray(w_W) * s[None, :] / N, dtype=f32)
    t2b = np.ascontiguousarray(np.broadcast_to(t2, (128, C)), dtype=f32)
    eye = np.eye(C, dtype=f32)
    id2 = np.ascontiguousarray(np.concatenate([eye[:128, :], eye[128:, :]], axis=1))
    return wphi_p, wg_p, wtt, wwf, t2b, id2


def _reference_fallback(x, w_g, b_g, w_theta, b_theta, w_phi, b_phi, w_W, b_W,
                        bn_gamma, bn_beta, bn_mean, bn_var):
    b, h, w, c = x.shape
    n = h * w
    xf = x.reshape(b, n, c).astype(np.float32)
    g_x = xf @ w_g + b_g
    theta_x = xf @ w_theta + b_theta
    phi_x = xf @ w_phi + b_phi
    a = np.einsum("bnd,bne->bde", phi_x, g_x) / n
    y = theta_x @ a
    w_y = y @ w_W + b_W
    w_y = bn_gamma * (w_y - bn_mean) / np.sqrt(bn_var + BN_EPS) + bn_beta
    return (w_y.reshape(b, h, w, c) + x).astype(np.float32)


def run_sharded(x, folded, f32r_big=True, trace=False):
    from concourse.bass_utils import run_bass_kernel_spmd

    nc = _get_nc(f32r_big)
    wphi_p, wg_p, wtt, wwf, t2b, id2 = folded
    xr = np.ascontiguousarray(np.asarray(x), dtype=np.float32).reshape(B, N, C)
    in_maps = [
        {"x": xr[i], "wphi": wphi_p, "wg": wg_p, "wtt": wtt, "wwf": wwf,
         "t2b": t2b, "id2": id2}
        for i in range(B)
    ]
    res = run_bass_kernel_spmd(nc, in_maps, list(range(B)), trace=trace)
    z = np.stack([res.results[i]["z"] for i in range(B)], axis=0)
    return z.reshape(B, H, W, C), res


def kernel(x, w_g, b_g, w_theta, b_theta, w_phi, b_phi, w_W, b_W,
           bn_gamma, bn_beta, bn_mean, bn_var):
    args = dict(w_g=np.asarray(w_g), b_g=np.asarray(b_g),
                w_theta=np.asarray(w_theta), b_theta=np.asarray(b_theta),
                w_phi=np.asarray(w_phi), b_phi=np.asarray(b_phi),
                w_W=np.asarray(w_W), b_W=np.asarray(b_W),
                bn_gamma=np.asarray(bn_gamma), bn_beta=np.asarray(bn_beta),
                bn_mean=np.asarray(bn_mean), bn_var=np.asarray(bn_var))
    x = np.asarray(x)
    # the device path folds the (zero) projection biases away; anything else
    # (never produced by setup_inputs) gets the exact host fallback
    if (np.any(args["b_g"]) or np.any(args["b_theta"]) or np.any(args["b_phi"])
            or x.shape != (B, H, W, C)):
        return _reference_fallback(x, **{k: v for k, v in args.items()})
    folded = _fold_params(**args)
    z, _ = run_sharded(x, folded)
    return z


# revision 7
# speedup vs baseline: 1.3095x; 1.2870x over previous
"""Non-local block (no softmax) on 8 Trainium2 cores, data-parallel over batch.

Math: per sample X [N=4096, C=256] (N = 64*64 spatial, C channels):
    theta = X Wt, phi = X Wp, g = X Wg          (biases are zero)
    y = (theta phi^T / N) g  ->  associativity (no softmax):
      y = X Wt A / N,  A = Wp^T G Wg,  G = X^T X
    z = y (Ww * s) + t2 + X  =  X (M2 + I) + t2
      M2 = Wt (A/N) (Ww * s),  s = gamma*rsqrt(var+eps),
      t2 = (b_W - mean)*s + beta
so the whole block is: G = X^T X (device), small 256x256 chain (device),
z = X (M2+I) + t2 (device). One sample per NeuronCore.

mode="f32r": x kept fp32, big matmuls in float32r.
mode="bf16": x cast to bf16 on host (halves input DMA; FWL halves
             LDWEIGHTS); G/transposes/z-matmuls in bf16, chain in f32r.
"""

import numpy as np
import ml_dtypes

B, H, W, C = 8, 64, 64, 256
IC = C // 2
N = H * W
NCHUNK = N // 128  # 32
BN_EPS = 1e-3

_CACHE = {}
DEFAULT_MODE = "bf16"


def _build_nc(mode: str):
    import concourse.bacc as bacc
    import concourse.mybir as mybir
    import concourse.tile as tile

    F32 = mybir.dt.float32
    F32R = mybir.dt.float32r
    BF16 = mybir.dt.bfloat16
    bf = mode == "bf16"
    XDT = BF16 if bf else F32

    def rc(ap):
        return ap.bitcast(F32R)

    # cast for the x-path (G matmuls, transposes, z matmuls)
    xc_ = (lambda ap: ap) if bf else rc

    nc = bacc.Bacc("TRN2", target_bir_lowering=False, debug=False)

    x_d = nc.dram_tensor("x", [N, C], XDT, kind="ExternalInput")
    wphi_d = nc.dram_tensor("wphi", [128, 256], F32, kind="ExternalInput")
    wg_d = nc.dram_tensor("wg", [128, 256], F32, kind="ExternalInput")
    wtt_d = nc.dram_tensor("wtt", [128, 256], F32, kind="ExternalInput")
    wwf_d = nc.dram_tensor("wwf", [128, 256], F32, kind="ExternalInput")
    t2b_d = nc.dram_tensor("t2b", [128, 256], F32, kind="ExternalInput")
    id2_d = nc.dram_tensor("id2", [128, 512], F32, kind="ExternalInput")
    idb_d = nc.dram_tensor("idb", [128, 128], XDT, kind="ExternalInput")
    z_d = nc.dram_tensor("z", [N, C], F32, kind="ExternalOutput")

    with tile.TileContext(nc) as tc:
        with (
            tc.tile_pool(name="const", bufs=1) as cpool,
            tc.tile_pool(name="big", bufs=1) as bpool,
            tc.tile_pool(name="zs", bufs=8) as zpool,
            tc.tile_pool(name="psg", bufs=1, space="PSUM") as psg,
            tc.tile_pool(name="psw", bufs=6, space="PSUM") as psw,
        ):
            wphi = cpool.tile([128, 256], F32, tag="wphi")
            wg = cpool.tile([128, 256], F32, tag="wg")
            wtt = cpool.tile([128, 256], F32, tag="wtt")
            wwf = cpool.tile([128, 256], F32, tag="wwf")
            t2b = cpool.tile([128, 256], F32, tag="t2b")
            id2 = cpool.tile([128, 512], F32, tag="id2")
            idb = cpool.tile([128, 128], XDT, tag="idb")
            # identity needed by the first transposes: load it first
            nc.sync.dma_start(xc_(idb[:]), xc_(idb_d[:]))
            nc.gpsimd.dma_start(rc(id2[:]), rc(id2_d[:]))

            x_nat = bpool.tile([128, NCHUNK * 256], XDT, tag="x_nat")
            xt0 = bpool.tile([128, N], XDT, tag="xt0")
            xt1 = bpool.tile([128, N], XDT, tag="xt1")

            identr = rc(id2[:, 0:128])  # eye(128) f32r view (chain transposes)

            # ---- phase 1: load x, G = X^T X, and build X^T via PE transpose
            g0 = psg.tile([128, 256], F32, tag="g0")
            g1 = psg.tile([128, 256], F32, tag="g1")
            for t in range(NCHUNK):
                xc = x_nat[:, t * 256 : (t + 1) * 256]
                xcl = x_nat[:, t * 256 : t * 256 + 128]
                xch = x_nat[:, t * 256 + 128 : (t + 1) * 256]
                dma_eng = nc.sync if t % 2 == 0 else nc.gpsimd
                dma_eng.dma_start(xc_(xc), xc_(x_d[t * 128 : (t + 1) * 128, :]))
                if t == 3:
                    # weights are consumed mid-kernel; slot their DMAs in
                    # after the pipeline is primed
                    nc.gpsimd.dma_start(rc(wphi[:]), rc(wphi_d[:]))
                    nc.sync.dma_start(rc(wg[:]), rc(wg_d[:]))
                    nc.gpsimd.dma_start(rc(wtt[:]), rc(wtt_d[:]))
                    nc.sync.dma_start(rc(wwf[:]), rc(wwf_d[:]))
                    nc.gpsimd.dma_start(t2b[:], t2b_d[:])
                nc.tensor.matmul(
                    g0[:], xc_(xcl), xc_(xc),
                    start=(t == 0), stop=(t == NCHUNK - 1),
                )
                nc.tensor.matmul(
                    g1[:], xc_(xch), xc_(xc),
                    start=(t == 0), stop=(t == NCHUNK - 1),
                )
                tp0 = psw.tile([128, 128], XDT, tag="w")
                nc.tensor.transpose(xc_(tp0[:]), xc_(xcl), xc_(idb[:]))
                tp1 = psw.tile([128, 128], XDT, tag="w")
                nc.tensor.transpose(xc_(tp1[:]), xc_(xch), xc_(idb[:]))
                dst0 = xc_(xt0[:, t * 128 : (t + 1) * 128])
                dst1 = xc_(xt1[:, t * 128 : (t + 1) * 128])
                # split PSUM->SBUF copies across DVE and ACT
                if t % 2 == 0:
                    nc.vector.tensor_copy(dst0, xc_(tp0[:]))
                    nc.scalar.copy(dst1, xc_(tp1[:]))
                else:
                    nc.scalar.copy(dst0, xc_(tp0[:]))
                    nc.vector.tensor_copy(dst1, xc_(tp1[:]))

            # ---- phase 2: small chain (f32r)
            g_s = bpool.tile([128, 512], F32, tag="g_s")
            nc.vector.tensor_copy(rc(g_s[:, 0:256]), g0[:])
            nc.vector.tensor_copy(rc(g_s[:, 256:512]), g1[:])

            pp1 = psw.tile([128, 256], F32, tag="w")
            nc.tensor.matmul(pp1[:], rc(wphi[:, 0:128]), rc(g_s[:, 0:256]),
                             start=True, stop=False)
            nc.tensor.matmul(pp1[:], rc(wphi[:, 128:256]), rc(g_s[:, 256:512]),
                             start=False, stop=True)
            p1_s = bpool.tile([128, 256], F32, tag="p1_s")
            nc.vector.tensor_copy(rc(p1_s[:]), pp1[:])

            p1t_s = bpool.tile([128, 256], F32, tag="p1t_s")
            for j in range(2):
                tp = psw.tile([128, 128], F32, tag="w")
                nc.tensor.transpose(rc(tp[:]),
                                    rc(p1_s[:, j * 128 : (j + 1) * 128]),
                                    identr)
                nc.vector.tensor_copy(rc(p1t_s[:, j * 128 : (j + 1) * 128]),
                                      rc(tp[:]))

            pa = psw.tile([128, 128], F32, tag="w")
            nc.tensor.matmul(pa[:], rc(p1t_s[:, 0:128]), rc(wg[:, 0:128]),
                             start=True, stop=False)
            nc.tensor.matmul(pa[:], rc(p1t_s[:, 128:256]), rc(wg[:, 128:256]),
                             start=False, stop=True)
            a_s = bpool.tile([128, 128], F32, tag="a_s")
            nc.vector.tensor_copy(rc(a_s[:]), pa[:])

            t1_s = bpool.tile([128, 256], F32, tag="t1_s")
            for j in range(2):
                pt = psw.tile([128, 128], F32, tag="w")
                nc.tensor.matmul(pt[:], rc(wtt[:, j * 128 : (j + 1) * 128]),
                                 rc(a_s[:]), start=True, stop=True)
                nc.vector.tensor_copy(rc(t1_s[:, j * 128 : (j + 1) * 128]), pt[:])

            t1t_s = bpool.tile([128, 256], F32, tag="t1t_s")
            for j in range(2):
                tp = psw.tile([128, 128], F32, tag="w")
                nc.tensor.transpose(rc(tp[:]),
                                    rc(t1_s[:, j * 128 : (j + 1) * 128]),
                                    identr)
                nc.vector.tensor_copy(rc(t1t_s[:, j * 128 : (j + 1) * 128]),
                                      rc(tp[:]))

            m2_s = bpool.tile([128, 512], XDT, tag="m2_s")
            for j in range(2):
                pm = psw.tile([128, 256], F32, tag="w")
                nc.tensor.matmul(pm[:], rc(t1t_s[:, j * 128 : (j + 1) * 128]),
                                 rc(wwf[:]), start=True, stop=True)
                # M2' = M2 + I  (adds the residual path)
                nc.vector.tensor_add(xc_(m2_s[:, j * 256 : (j + 1) * 256]), pm[:],
                                     id2[:, j * 256 : (j + 1) * 256])

            # ---- phase 3: z = X M2' + t2
            for t in range(NCHUNK):
                pz = psw.tile([128, 256], F32, tag="w")
                nc.tensor.matmul(pz[:], xc_(xt0[:, t * 128 : (t + 1) * 128]),
                                 xc_(m2_s[:, 0:256]), start=True, stop=False)
                nc.tensor.matmul(pz[:], xc_(xt1[:, t * 128 : (t + 1) * 128]),
                                 xc_(m2_s[:, 256:512]), start=False, stop=True)
                z_s = zpool.tile([128, 256], F32, tag="z")
                nc.vector.tensor_add(z_s[:], pz[:], t2b[:])
                (nc.sync if t % 2 == 0 else nc.gpsimd).dma_start(
                    z_d[t * 128 : (t + 1) * 128, :], z_s[:])

    nc.compile()
    return nc


def _get_nc(mode=DEFAULT_MODE):
    key = ("nc", mode)
    if key not in _CACHE:
        _CACHE[key] = _build_nc(mode)
    return _CACHE[key]


def _fold_params(w_g, b_g, w_theta, b_theta, w_phi, b_phi, w_W, b_W,
                 bn_gamma, bn_beta, bn_mean, bn_var):
    f32 = np.float32
    s = (bn_gamma / np.sqrt(bn_var + BN_EPS)).astype(f32)
    t2 = ((b_W - bn_mean) * s + bn_beta).astype(f32)
    pack = lambda w: np.ascontiguousarray(
        np.concatenate([w[:128, :], w[128:, :]], axis=1), dtype=f32)
    wphi_p = pack(np.asarray(w_phi))
    wg_p = pack(np.asarray(w_g))
    wtt = np.ascontiguousarray(np.asarray(w_theta).T, dtype=f32)
    wwf = np.ascontiguousarray(np.asarray(w_W) * s[None, :] / N, dtype=f32)
    t2b = np.ascontiguousarray(np.broadcast_to(t2, (128, C)), dtype=f32)
    eye = np.eye(C, dtype=f32)
    id2 = np.ascontiguousarray(np.concatenate([eye[:128, :], eye[128:, :]], axis=1))
    return wphi_p, wg_p, wtt, wwf, t2b, id2


def _reference_fallback(x, w_g, b_g, w_theta, b_theta, w_phi, b_phi, w_W, b_W,
                        bn_gamma, bn_beta, bn_mean, bn_var):
    b, h, w, c = x.shape
    n = h * w
    xf = x.reshape(b, n, c).astype(np.float32)
    g_x = xf @ w_g + b_g
    theta_x = xf @ w_theta + b_theta
    phi_x = xf @ w_phi + b_phi
    a = np.einsum("bnd,bne->bde", phi_x, g_x) / n
    y = theta_x @ a
    w_y = y @ w_W + b_W
    w_y = bn_gamma * (w_y - bn_mean) / np.sqrt(bn_var + BN_EPS) + bn_beta
    return (w_y.reshape(b, h, w, c) + x).astype(np.float32)


def run_sharded(x, folded, mode=DEFAULT_MODE, trace=False):
    from concourse.bass_utils import run_bass_kernel_spmd

    nc = _get_nc(mode)
    wphi_p, wg_p, wtt, wwf, t2b, id2 = folded
    xdt = ml_dtypes.bfloat16 if mode == "bf16" else np.float32
    xr = np.ascontiguousarray(
        np.asarray(x, dtype=np.float32).reshape(B, N, C).astype(xdt))
    idb = np.eye(128, dtype=xdt)
    in_maps = [
        {"x": xr[i], "wphi": wphi_p, "wg": wg_p, "wtt": wtt, "wwf": wwf,
         "t2b": t2b, "id2": id2, "idb": idb}
        for i in range(B)
    ]
    res = run_bass_kernel_spmd(nc, in_maps, list(range(B)), trace=trace)
    z = np.stack([res.results[i]["z"] for i in range(B)], axis=0)
    return z.reshape(B, H, W, C), res


def kernel(x, w_g, b_g, w_theta, b_theta, w_phi, b_phi, w_W, b_W,
           bn_gamma, bn_beta, bn_mean, bn_var):
    args = dict(w_g=np.asarray(w_g), b_g=np.asarray(b_g),
                w_theta=np.asarray(w_theta), b_theta=np.asarray(b_theta),
                w_phi=np.asarray(w_phi), b_phi=np.asarray(b_phi),
                w_W=np.asarray(w_W), b_W=np.asarray(b_W),
                bn_gamma=np.asarray(bn_gamma), bn_beta=np.asarray(bn_beta),
                bn_mean=np.asarray(bn_mean), bn_var=np.asarray(bn_var))
    x = np.asarray(x)
    # the device path folds the (zero) projection biases away; anything else
    # (never produced by setup_inputs) gets the exact host fallback
    if (np.any(args["b_g"]) or np.any(args["b_theta"]) or np.any(args["b_phi"])
            or x.shape != (B, H, W, C)):
        return _reference_fallback(x, **{k: v for k, v in args.items()})
    folded = _fold_params(**args)
    z, _ = run_sharded(x, folded)
    return z
